# revision 32
# baseline (speedup 1.0000x reference)
"""Causal self-attention (QKV proj + RoPE + causal softmax attention + out proj)
for Trainium2, distributed over 8 NeuronCores.

Sharding: 4 batches x 2 head-groups (tensor parallel over heads within a batch).
Each core computes, for its (batch b, head-group g of 8 heads):
  - qkv = x[b] @ w_qkv[:, cols(g)]   (feature-major via lhsT = w chunks)
  - RoPE on q, k
  - causal softmax attention for its 8 heads (transposed-scores layout)
  - out_partial = att @ w_proj[rows(g), :]
Host gathers: out[b] = partial(b,0) + partial(b,1)  (the 2-way TP all-reduce),
and reassembles k, v from per-core feature-major slices.

All matmuls run in float32r (fp32 data, hardware rounds operands to 11 mantissa
bits, ~bf16 throughput at N>=512).
"""

import math

import numpy as np

import concourse.bass as bass
import concourse.tile as tile
import concourse.mybir as mybir
from concourse import bacc
from concourse.bass_utils import run_bass_kernel_spmd

F32 = mybir.dt.float32
F32R = mybir.dt.float32r
BF16 = mybir.dt.bfloat16
AF = mybir.ActivationFunctionType

B, T, C = 4, 2048, 2048
N_HEAD = 16
D = C // N_HEAD          # 128
HALF = D // 2            # 64
HPG = 8                  # heads per group (2 groups)
ROPE_BASE = 10000.0
ISQRT_D = 1.0 / math.sqrt(D)

NCTILE = C // 128        # 16 contraction tiles
NQB = T // 512           # 4 query blocks of 512
NKB = T // 128           # 16 key blocks of 128


def _build_program():
    nc = bacc.Bacc("TRN2", target_bir_lowering=False, debug=False, num_devices=8)

    # per-core inputs
    xT_d = nc.dram_tensor("xT", [NCTILE, 128, T], F32R, kind="ExternalInput").ap()
    wqkv_d = nc.dram_tensor("wqkv", [C, 3 * HPG * D], F32R, kind="ExternalInput").ap()
    wproj_d = nc.dram_tensor("wproj", [HPG * D, C], F32R, kind="ExternalInput").ap()
    ccat_d = nc.dram_tensor("ccat", [128, T], F32, kind="ExternalInput").ap()
    scat_d = nc.dram_tensor("scat", [128, T], F32, kind="ExternalInput").ap()
    mask_d = nc.dram_tensor("trimask", [128, 128], BF16, kind="ExternalInput").ap()

    # per-core outputs
    out_d = nc.dram_tensor("out_part", [T, C], F32, kind="ExternalOutput").ap()
    k_out_d = nc.dram_tensor("k_out", [HPG, D, T], F32, kind="ExternalOutput").ap()
    v_out_d = nc.dram_tensor("v_out", [HPG, D, T], F32, kind="ExternalOutput").ap()

    wqkv_r = wqkv_d.rearrange("(c p) f -> p c f", p=128)    # [128, 16, 3072]
    xT_r = xT_d.rearrange("c p t -> p c t")                 # [128, 16, 2048]
    wproj_r = wproj_d.rearrange("(h p) c -> p h c", p=128)  # [128, 8, 2048]

    with tile.TileContext(nc) as tc:
        with tc.tile_pool(name="dram", bufs=1, space="DRAM") as dpool, \
             tc.tile_pool(name="const", bufs=1) as constp:

            # per-head scratch: head h's reload only depends on head h's writes
            q_scr = [dpool.tile([128, T], F32R, tag=f"q_scr{h}",
                                name=f"q_scr{h}") for h in range(HPG)]
            k_scr = [dpool.tile([128, T], F32R, tag=f"k_scr{h}",
                                name=f"k_scr{h}") for h in range(HPG)]
            v_scr = [dpool.tile([128, T], F32R, tag=f"v_scr{h}",
                                name=f"v_scr{h}") for h in range(HPG)]

            mask_sb = constp.tile([128, 128], BF16, tag="mask")
            nc.sync.dma_start(out=mask_sb[:], in_=mask_d)
            onesm_f = constp.tile([128, 128], F32, tag="onesmf")
            nc.vector.memset(onesm_f[:], 1.0)
            onesm = constp.tile([128, 128], F32R, tag="onesm")
            nc.vector.tensor_copy(onesm[:], onesm_f[:])
            ident_f = constp.tile([128, 128], F32, tag="ident_f")
            from concourse.masks import make_identity
            make_identity(nc, ident_f[:])
            ident = constp.tile([128, 128], F32R, tag="ident")
            nc.vector.tensor_copy(ident[:], ident_f[:])

            # ---------------- Phase 1: QKV projection + RoPE ----------------
            with tc.tile_pool(name="xh", bufs=1) as xp, \
                 tc.tile_pool(name="wq", bufs=3) as wp, \
                 tc.tile_pool(name="qkvps", bufs=8, space="PSUM") as qkvps, \
                 tc.tile_pool(name="stag", bufs=3) as stp, \
                 tc.tile_pool(name="rope", bufs=2) as rp, \
                 tc.tile_pool(name="cs", bufs=1) as csp:

                # stacked rope tables: ccat = [cos; cos], scat = [-sin; sin]
                ccat_sb = csp.tile([128, T], F32, tag="ccat")
                nc.sync.dma_start(out=ccat_sb[:], in_=ccat_d)
                scat_sb = csp.tile([128, T], F32, tag="scat")
                nc.sync.dma_start(out=scat_sb[:], in_=scat_d)

                xh = xp.tile([128, NCTILE, T], F32R, tag="xh")
                nc.sync.dma_start(out=xh[:], in_=xT_r)

                def rope(ps, tb, out_tile):
                    # out = ps*ccat + rot(ps)*scat, rot = [x2; x1], scat = [-s; s]
                    ts_ = slice(tb * 512, (tb + 1) * 512)
                    rot = rp.tile([128, 512], F32, tag="rot")
                    nc.vector.tensor_copy(rot[0:HALF, :], ps[HALF:128, :])
                    nc.vector.tensor_copy(rot[HALF:128, :], ps[0:HALF, :])
                    p1 = rp.tile([128, 512], F32, tag="p1")
                    nc.vector.tensor_mul(p1[:], ps[:], ccat_sb[:, ts_])
                    p2 = rp.tile([128, 512], F32, tag="p2")
                    nc.vector.tensor_mul(p2[:], rot[:], scat_sb[:, ts_])
                    nc.vector.tensor_add(out_tile[:], p1[:], p2[:])

                for fb in range(3 * HPG):
                    w_t = wp.tile([128, NCTILE, 128], F32R, tag="w")
                    nc.sync.dma_start(
                        out=w_t[:], in_=wqkv_r[:, :, fb * 128:(fb + 1) * 128])
                    pss = [qkvps.tile([128, 512], F32, tag="qkvps",
                                      name=f"qkvps_{fb}_{i}")
                           for i in range(NQB)]
                    for ci in range(NCTILE):
                        for tb in range(NQB):
                            nc.tensor.matmul(
                                pss[tb][:], w_t[:, ci, :],
                                xh[:, ci, tb * 512:(tb + 1) * 512],
                                start=(ci == 0), stop=(ci == NCTILE - 1))
                    for tb in range(NQB):
                        ps = pss[tb]
                        ts_ = slice(tb * 512, (tb + 1) * 512)
                        if fb < HPG:           # q
                            h = fb
                            qf = stp.tile([128, 512], F32R, tag="qstag")
                            rope(ps, tb, qf)
                            nc.sync.dma_start(out=q_scr[h][:, ts_], in_=qf[:])
                        elif fb < 2 * HPG:     # k
                            h = fb - HPG
                            kf = stp.tile([128, 512], F32, tag="kstag")
                            rope(ps, tb, kf)
                            nc.sync.dma_start(out=k_out_d[h, :, ts_], in_=kf[:])
                            nc.sync.dma_start(out=k_scr[h][:, ts_],
                                              in_=kf[:].bitcast(F32R))
                        else:                  # v
                            h = fb - 2 * HPG
                            vf = stp.tile([128, 512], F32, tag="vstag")
                            nc.vector.tensor_copy(vf[:], ps[:])
                            nc.sync.dma_start(out=v_out_d[h, :, ts_], in_=vf[:])
                            nc.sync.dma_start(out=v_scr[h][:, ts_],
                                              in_=vf[:].bitcast(F32R))

            # ---------------- Phase 2+3: attention, then projection ----------
            with tc.tile_pool(name="att", bufs=1) as attp, \
                 tc.tile_pool(name="wp", bufs=1) as wpp:
                att_all = attp.tile([128, HPG, T], F32R, tag="att_all")
                # w_proj tile; prefetched mid-attention (after head 1 starts)
                # so it doesn't delay head 0's q/k/v loads
                wp_t = wpp.tile([128, HPG, C], F32R, tag="wp")

                with tc.tile_pool(name="qh", bufs=2) as qhp, \
                     tc.tile_pool(name="kh", bufs=2) as khp, \
                     tc.tile_pool(name="vh", bufs=2) as vhp, \
                     tc.tile_pool(name="vtok", bufs=2) as vtp, \
                     tc.tile_pool(name="vtps", bufs=1, space="PSUM") as vtps, \
                     tc.tile_pool(name="sps", bufs=3, space="PSUM") as sps, \
                     tc.tile_pool(name="sumps", bufs=1, space="PSUM") as sumps, \
                     tc.tile_pool(name="avps", bufs=2, space="PSUM") as avps, \
                     tc.tile_pool(name="probs", bufs=4) as prp, \
                     tc.tile_pool(name="rcp", bufs=2) as rcp:

                    for h in range(HPG):
                        if h == 2:
                            nc.sync.dma_start(out=wp_t[:], in_=wproj_r)
                        q_t = qhp.tile([128, T], F32R, tag="qh")
                        nc.scalar.dma_start(out=q_t[:], in_=q_scr[h][:])
                        k_t = khp.tile([128, T], F32R, tag="kh")
                        nc.scalar.dma_start(out=k_t[:], in_=k_scr[h][:])
                        v_h = vhp.tile([128, T], F32R, tag="vh")
                        nc.scalar.dma_start(out=v_h[:], in_=v_scr[h][:])
                        vtok = vtp.tile([128, NKB, 128], F32R, tag="vtok")
                        for tbk in range(NKB):
                            pvt = vtps.tile([128, 128], F32R, tag="vtps")
                            nc.tensor.transpose(
                                pvt[:], v_h[:, tbk * 128:(tbk + 1) * 128],
                                ident[:])
                            nc.vector.tensor_copy(vtok[:, tbk, :], pvt[:])

                        for qi in range(NQB):
                            nkb = 4 * qi + 4
                            qs = slice(qi * 512, (qi + 1) * 512)
                            # all-ones lhsT: sum over keys, broadcast to all rows
                            ps_sum = sumps.tile([128, 512], F32, tag="ps_sum")
                            ps_o = avps.tile([128, 512], F32, tag="ps_o")
                            pending = []

                            def sum_av(j, pr, left, *, _sum=ps_sum, _o=ps_o,
                                       _nkb=nkb):
                                cs_ = slice(left, 512)
                                nc.tensor.matmul(
                                    _sum[:, cs_], onesm[:], pr[:, cs_],
                                    start=(j == 0), stop=(j == _nkb - 1))
                                nc.tensor.matmul(
                                    _o[:, cs_], vtok[:, j, :], pr[:, cs_],
                                    start=(j == 0), stop=(j == _nkb - 1))

                            for kb in range(nkb):
                                ps_s = sps.tile([128, 512], F32, tag="ps_s")
                                nc.tensor.matmul(
                                    ps_s[:], k_t[:, kb * 128:(kb + 1) * 128],
                                    q_t[:, qs], start=True, stop=True)
                                m = kb - 4 * qi
                                left = max(m, 0) * 128
                                pr = prp.tile([128, 512], F32R, tag="probs")
                                nc.scalar.activation(
                                    pr[:, left:], ps_s[:, left:], AF.Exp,
                                    scale=ISQRT_D)
                                if m >= 0:
                                    dg = slice(left, left + 128)
                                    nc.vector.tensor_mul(
                                        pr[:, dg], pr[:, dg], mask_sb[:])
                                pending.append((kb, pr, left))
                                if len(pending) > 2:
                                    sum_av(*pending.pop(0))
                            while pending:
                                sum_av(*pending.pop(0))

                            rbc = rcp.tile([128, 512], F32, tag="rbc")
                            nc.vector.reciprocal_approx_fast(
                                out=rbc[:], in_=ps_sum[:])
                            nc.vector.tensor_mul(att_all[:, h, qs], ps_o[:], rbc[:])

                # ---------------- Phase 3: output projection ----------------
                # w_proj already resident; lhsT (att tile) reused across 4 cb
                with tc.tile_pool(name="pps", bufs=8, space="PSUM") as pps, \
                     tc.tile_pool(name="ostag", bufs=4) as osp:
                    for tb in range(NKB):
                        psp = [pps.tile([128, 512], F32, tag="ps_p",
                                        name=f"ps_p_{tb}_{i}")
                               for i in range(4)]
                        for hh in range(HPG):
                            for cb in range(4):
                                nc.tensor.matmul(
                                    psp[cb][:],
                                    att_all[:, hh, tb * 128:(tb + 1) * 128],
                                    wp_t[:, hh, cb * 512:(cb + 1) * 512],
                                    start=(hh == 0), stop=(hh == HPG - 1))
                        for cb in range(4):
                            o_sb = osp.tile([128, 512], F32, tag="ostag",
                                            name=f"o_sb_{tb}_{cb}")
                            nc.vector.tensor_copy(o_sb[:], psp[cb][:])
                            nc.sync.dma_start(
                                out=out_d[tb * 128:(tb + 1) * 128,
                                          cb * 512:(cb + 1) * 512],
                                in_=o_sb[:])

    nc.compile()
    return nc


_NC_CACHE = None


def _get_program():
    global _NC_CACHE
    if _NC_CACHE is None:
        _NC_CACHE = _build_program()
    return _NC_CACHE


def _host_prep(x, w_qkv, w_proj):
    """Build the 8 per-core input maps (core = b*2 + g)."""
    x = np.ascontiguousarray(np.asarray(x, dtype=np.float32))
    w_qkv = np.asarray(w_qkv, dtype=np.float32)
    w_proj = np.asarray(w_proj, dtype=np.float32)

    # rope tables, feature-major [HALF, T]
    inv_freq = 1.0 / (ROPE_BASE ** (np.arange(HALF, dtype=np.float32) / HALF))
    pos = np.arange(T, dtype=np.float32)
    freqs = pos[:, None] * inv_freq[None, :]          # [T, HALF]
    cos_t = np.cos(freqs).T.astype(np.float32)      # [HALF, T]
    sin_t = np.sin(freqs).T.astype(np.float32)
    ccat = np.ascontiguousarray(np.concatenate([cos_t, cos_t], axis=0))
    scat = np.ascontiguousarray(np.concatenate([-sin_t, sin_t], axis=0))

    import ml_dtypes
    tri = np.triu(np.ones((128, 128), dtype=np.float32))  # [k,q]: 1 if k<=q
    tri = tri.astype(ml_dtypes.bfloat16)

    in_maps = []
    for core in range(8):
        b, g = divmod(core, 2)
        xT = np.ascontiguousarray(x[b].T).reshape(NCTILE, 128, T)
        cols = []
        for part in range(3):                          # q, k, v column groups
            base = part * C + g * HPG * D
            cols.append(w_qkv[:, base:base + HPG * D])
        wq = np.ascontiguousarray(np.concatenate(cols, axis=1))
        wpj = np.ascontiguousarray(w_proj[g * HPG * D:(g + 1) * HPG * D, :])
        in_maps.append({
            "xT": xT, "wqkv": wq, "wproj": wpj,
            "ccat": ccat, "scat": scat, "trimask": tri,
        })
    return in_maps


def kernel(x, w_qkv, w_proj):
    nc = _get_program()
    in_maps = _host_prep(x, w_qkv, w_proj)
    res = run_bass_kernel_spmd(nc, in_maps, list(range(8)))

    out = np.empty((B, T, C), dtype=np.float32)
    k = np.empty((B, N_HEAD, T, D), dtype=np.float32)
    v = np.empty((B, N_HEAD, T, D), dtype=np.float32)
    for b in range(B):
        r0 = res.results[2 * b]
        r1 = res.results[2 * b + 1]
        out[b] = r0["out_part"] + r1["out_part"]
        for g, r in ((0, r0), (1, r1)):
            hs = slice(g * HPG, (g + 1) * HPG)
            k[b, hs] = np.transpose(r["k_out"], (0, 2, 1))
            v[b, hs] = np.transpose(r["v_out"], (0, 2, 1))
    return out, k, v


# revision 34
# speedup vs baseline: 1.0936x; 1.0936x over previous
"""Causal self-attention (QKV proj + RoPE + causal softmax attention + out proj)
for Trainium2, distributed over 8 NeuronCores.

Sharding: 4 batches x 2 head-groups (tensor parallel over heads within a batch).
Each core computes, for its (batch b, head-group g of 8 heads):
  - qkv = x[b] @ w_qkv[:, cols(g)]   (feature-major via lhsT = w chunks)
  - RoPE on q, k
  - causal softmax attention for its 8 heads (transposed-scores layout)
  - out_partial = att @ w_proj[rows(g), :]
Host gathers: out[b] = partial(b,0) + partial(b,1)  (the 2-way TP all-reduce),
and reassembles k, v from per-core feature-major slices.

All matmuls run in float32r (fp32 data, hardware rounds operands to 11 mantissa
bits, ~bf16 throughput at N>=512).
"""

import math

import numpy as np

import concourse.bass as bass
import concourse.tile as tile
import concourse.mybir as mybir
from concourse import bacc
from concourse.bass_utils import run_bass_kernel_spmd

F32 = mybir.dt.float32
F32R = mybir.dt.float32r
BF16 = mybir.dt.bfloat16
AF = mybir.ActivationFunctionType

B, T, C = 4, 2048, 2048
N_HEAD = 16
D = C // N_HEAD          # 128
HALF = D // 2            # 64
HPG = 8                  # heads per group (2 groups)
ROPE_BASE = 10000.0
ISQRT_D = 1.0 / math.sqrt(D)

NCTILE = C // 128        # 16 contraction tiles
NQB = T // 512           # 4 query blocks of 512
NKB = T // 128           # 16 key blocks of 128


def _build_program():
    nc = bacc.Bacc("TRN2", target_bir_lowering=False, debug=False, num_devices=8)

    # per-core inputs
    xT_d = nc.dram_tensor("xT", [NCTILE, 128, T], F32R, kind="ExternalInput").ap()
    wqkv_d = nc.dram_tensor("wqkv", [C, 3 * HPG * D], F32R, kind="ExternalInput").ap()
    wproj_d = nc.dram_tensor("wproj", [HPG * D, C], F32R, kind="ExternalInput").ap()
    ccat_d = nc.dram_tensor("ccat", [128, T], F32, kind="ExternalInput").ap()
    scat_d = nc.dram_tensor("scat", [128, T], F32, kind="ExternalInput").ap()
    mask_d = nc.dram_tensor("trimask", [128, 128], BF16, kind="ExternalInput").ap()

    # per-core outputs
    out_d = nc.dram_tensor("out_part", [T, C], F32, kind="ExternalOutput").ap()
    k_out_d = nc.dram_tensor("k_out", [HPG, D, T], F32, kind="ExternalOutput").ap()
    v_out_d = nc.dram_tensor("v_out", [HPG, D, T], F32, kind="ExternalOutput").ap()

    wqkv_r = wqkv_d.rearrange("(c p) f -> p c f", p=128)    # [128, 16, 3072]
    xT_r = xT_d.rearrange("c p t -> p c t")                 # [128, 16, 2048]
    wproj_r = wproj_d.rearrange("(h p) c -> p h c", p=128)  # [128, 8, 2048]

    with tile.TileContext(nc) as tc:
        with tc.tile_pool(name="dram", bufs=1, space="DRAM") as dpool, \
             tc.tile_pool(name="const", bufs=1) as constp:

            # per-head scratch: head h's reload only depends on head h's writes
            q_scr = [dpool.tile([128, T], F32R, tag=f"q_scr{h}",
                                name=f"q_scr{h}") for h in range(HPG)]
            k_scr = [dpool.tile([128, T], F32R, tag=f"k_scr{h}",
                                name=f"k_scr{h}") for h in range(HPG)]
            v_scr = [dpool.tile([128, T], F32R, tag=f"v_scr{h}",
                                name=f"v_scr{h}") for h in range(HPG)]

            mask_sb = constp.tile([128, 128], BF16, tag="mask")
            nc.sync.dma_start(out=mask_sb[:], in_=mask_d)
            onesm_f = constp.tile([128, 128], F32, tag="onesmf")
            nc.vector.memset(onesm_f[:], 1.0)
            onesm = constp.tile([128, 128], F32R, tag="onesm")
            nc.vector.tensor_copy(onesm[:], onesm_f[:])
            ident_f = constp.tile([128, 128], F32, tag="ident_f")
            from concourse.masks import make_identity
            make_identity(nc, ident_f[:])
            ident = constp.tile([128, 128], F32R, tag="ident")
            nc.vector.tensor_copy(ident[:], ident_f[:])

            # ---------------- Phase 1: QKV projection + RoPE ----------------
            with tc.tile_pool(name="xh", bufs=1) as xp, \
                 tc.tile_pool(name="wq", bufs=3) as wp, \
                 tc.tile_pool(name="qkvps", bufs=8, space="PSUM") as qkvps, \
                 tc.tile_pool(name="stag", bufs=3) as stp, \
                 tc.tile_pool(name="rope", bufs=2) as rp, \
                 tc.tile_pool(name="cs", bufs=1) as csp:

                # stacked rope tables: ccat = [cos; cos], scat = [-sin; sin]
                ccat_sb = csp.tile([128, T], F32, tag="ccat")
                nc.sync.dma_start(out=ccat_sb[:], in_=ccat_d)
                scat_sb = csp.tile([128, T], F32, tag="scat")
                nc.sync.dma_start(out=scat_sb[:], in_=scat_d)

                xh = xp.tile([128, NCTILE, T], F32R, tag="xh")
                nc.sync.dma_start(out=xh[:], in_=xT_r)

                def rope(ps, tb, out_tile):
                    # out = ps*ccat + rot(ps)*scat, rot = [x2; x1], scat = [-s; s]
                    ts_ = slice(tb * 512, (tb + 1) * 512)
                    rot = rp.tile([128, 512], F32, tag="rot")
                    nc.vector.tensor_copy(rot[0:HALF, :], ps[HALF:128, :])
                    nc.vector.tensor_copy(rot[HALF:128, :], ps[0:HALF, :])
                    p1 = rp.tile([128, 512], F32, tag="p1")
                    nc.vector.tensor_mul(p1[:], ps[:], ccat_sb[:, ts_])
                    p2 = rp.tile([128, 512], F32, tag="p2")
                    nc.vector.tensor_mul(p2[:], rot[:], scat_sb[:, ts_])
                    nc.vector.tensor_add(out_tile[:], p1[:], p2[:])

                for fb in range(3 * HPG):
                    w_t = wp.tile([128, NCTILE, 128], F32R, tag="w")
                    nc.sync.dma_start(
                        out=w_t[:], in_=wqkv_r[:, :, fb * 128:(fb + 1) * 128])
                    pss = [qkvps.tile([128, 512], F32, tag="qkvps",
                                      name=f"qkvps_{fb}_{i}")
                           for i in range(NQB)]
                    for ci in range(NCTILE):
                        for tb in range(NQB):
                            nc.tensor.matmul(
                                pss[tb][:], w_t[:, ci, :],
                                xh[:, ci, tb * 512:(tb + 1) * 512],
                                start=(ci == 0), stop=(ci == NCTILE - 1))
                    for tb in range(NQB):
                        ps = pss[tb]
                        ts_ = slice(tb * 512, (tb + 1) * 512)
                        if fb < HPG:           # q
                            h = fb
                            qf = stp.tile([128, 512], F32R, tag="qstag")
                            rope(ps, tb, qf)
                            nc.sync.dma_start(out=q_scr[h][:, ts_], in_=qf[:])
                        elif fb < 2 * HPG:     # k
                            h = fb - HPG
                            kf = stp.tile([128, 512], F32, tag="kstag")
                            rope(ps, tb, kf)
                            nc.sync.dma_start(out=k_out_d[h, :, ts_], in_=kf[:])
                            nc.sync.dma_start(out=k_scr[h][:, ts_],
                                              in_=kf[:].bitcast(F32R))
                        else:                  # v
                            h = fb - 2 * HPG
                            vf = stp.tile([128, 512], F32, tag="vstag")
                            nc.vector.tensor_copy(vf[:], ps[:])
                            nc.sync.dma_start(out=v_out_d[h, :, ts_], in_=vf[:])
                            nc.sync.dma_start(out=v_scr[h][:, ts_],
                                              in_=vf[:].bitcast(F32R))

            # ---------------- Phase 2+3: attention, then projection ----------
            with tc.tile_pool(name="att", bufs=1) as attp, \
                 tc.tile_pool(name="wp", bufs=1) as wpp:
                att_all = attp.tile([128, HPG, T], F32R, tag="att_all")
                # w_proj tile; prefetched mid-attention (after head 1 starts)
                # so it doesn't delay head 0's q/k/v loads
                wp_t = wpp.tile([128, HPG, C], F32R, tag="wp")

                with tc.tile_pool(name="qh", bufs=2) as qhp, \
                     tc.tile_pool(name="kh", bufs=2) as khp, \
                     tc.tile_pool(name="vh", bufs=2) as vhp, \
                     tc.tile_pool(name="vtok", bufs=2) as vtp, \
                     tc.tile_pool(name="vtps", bufs=1, space="PSUM") as vtps, \
                     tc.tile_pool(name="sps", bufs=3, space="PSUM") as sps, \
                     tc.tile_pool(name="sumps", bufs=1, space="PSUM") as sumps, \
                     tc.tile_pool(name="avps", bufs=2, space="PSUM") as avps, \
                     tc.tile_pool(name="probs", bufs=4) as prp, \
                     tc.tile_pool(name="rcp", bufs=2) as rcp:

                    for h in range(HPG):
                        if h == 2:
                            nc.sync.dma_start(out=wp_t[:], in_=wproj_r)
                        q_t = qhp.tile([128, T], F32R, tag="qh")
                        k_t = khp.tile([128, T], F32R, tag="kh")
                        v_h = vhp.tile([128, T], F32R, tag="vh")
                        if h < 2:
                            # schedule the first heads' reloads ahead of the
                            # QKV tail writes on the Sync queue
                            with tc.high_priority():
                                nc.sync.dma_start(out=q_t[:], in_=q_scr[h][:])
                                nc.sync.dma_start(out=k_t[:], in_=k_scr[h][:])
                                nc.sync.dma_start(out=v_h[:], in_=v_scr[h][:])
                        else:
                            nc.sync.dma_start(out=q_t[:], in_=q_scr[h][:])
                            nc.sync.dma_start(out=k_t[:], in_=k_scr[h][:])
                            nc.sync.dma_start(out=v_h[:], in_=v_scr[h][:])
                        vtok = vtp.tile([128, NKB, 128], F32R, tag="vtok")
                        for tbk in range(NKB):
                            pvt = vtps.tile([128, 128], F32R, tag="vtps")
                            nc.tensor.transpose(
                                pvt[:], v_h[:, tbk * 128:(tbk + 1) * 128],
                                ident[:])
                            nc.vector.tensor_copy(vtok[:, tbk, :], pvt[:])

                        for qi in range(NQB):
                            nkb = 4 * qi + 4
                            qs = slice(qi * 512, (qi + 1) * 512)
                            # all-ones lhsT: sum over keys, broadcast to all rows
                            ps_sum = sumps.tile([128, 512], F32, tag="ps_sum")
                            ps_o = avps.tile([128, 512], F32, tag="ps_o")
                            pending = []

                            def sum_av(j, pr, left, *, _sum=ps_sum, _o=ps_o,
                                       _nkb=nkb):
                                cs_ = slice(left, 512)
                                nc.tensor.matmul(
                                    _sum[:, cs_], onesm[:], pr[:, cs_],
                                    start=(j == 0), stop=(j == _nkb - 1))
                                nc.tensor.matmul(
                                    _o[:, cs_], vtok[:, j, :], pr[:, cs_],
                                    start=(j == 0), stop=(j == _nkb - 1))

                            for kb in range(nkb):
                                ps_s = sps.tile([128, 512], F32, tag="ps_s")
                                nc.tensor.matmul(
                                    ps_s[:], k_t[:, kb * 128:(kb + 1) * 128],
                                    q_t[:, qs], start=True, stop=True)
                                m = kb - 4 * qi
                                left = max(m, 0) * 128
                                pr = prp.tile([128, 512], F32R, tag="probs")
                                nc.scalar.activation(
                                    pr[:, left:], ps_s[:, left:], AF.Exp,
                                    scale=ISQRT_D)
                                if m >= 0:
                                    dg = slice(left, left + 128)
                                    nc.vector.tensor_mul(
                                        pr[:, dg], pr[:, dg], mask_sb[:])
                                pending.append((kb, pr, left))
                                if len(pending) > 2:
                                    sum_av(*pending.pop(0))
                            while pending:
                                sum_av(*pending.pop(0))

                            rbc = rcp.tile([128, 512], F32, tag="rbc")
                            nc.vector.reciprocal_approx_fast(
                                out=rbc[:], in_=ps_sum[:])
                            nc.vector.tensor_mul(att_all[:, h, qs], ps_o[:], rbc[:])

                # ---------------- Phase 3: output projection ----------------
                # w_proj already resident; lhsT (att tile) reused across 4 cb
                with tc.tile_pool(name="pps", bufs=8, space="PSUM") as pps, \
                     tc.tile_pool(name="ostag", bufs=4) as osp:
                    for tb in range(NKB):
                        psp = [pps.tile([128, 512], F32, tag="ps_p",
                                        name=f"ps_p_{tb}_{i}")
                               for i in range(4)]
                        for hh in range(HPG):
                            for cb in range(4):
                                nc.tensor.matmul(
                                    psp[cb][:],
                                    att_all[:, hh, tb * 128:(tb + 1) * 128],
                                    wp_t[:, hh, cb * 512:(cb + 1) * 512],
                                    start=(hh == 0), stop=(hh == HPG - 1))
                        for cb in range(4):
                            o_sb = osp.tile([128, 512], F32, tag="ostag",
                                            name=f"o_sb_{tb}_{cb}")
                            nc.vector.tensor_copy(o_sb[:], psp[cb][:])
                            nc.sync.dma_start(
                                out=out_d[tb * 128:(tb + 1) * 128,
                                          cb * 512:(cb + 1) * 512],
                                in_=o_sb[:])

    nc.compile()
    return nc


_NC_CACHE = None


def _get_program():
    global _NC_CACHE
    if _NC_CACHE is None:
        _NC_CACHE = _build_program()
    return _NC_CACHE


def _host_prep(x, w_qkv, w_proj):
    """Build the 8 per-core input maps (core = b*2 + g)."""
    x = np.ascontiguousarray(np.asarray(x, dtype=np.float32))
    w_qkv = np.asarray(w_qkv, dtype=np.float32)
    w_proj = np.asarray(w_proj, dtype=np.float32)

    # rope tables, feature-major [HALF, T]
    inv_freq = 1.0 / (ROPE_BASE ** (np.arange(HALF, dtype=np.float32) / HALF))
    pos = np.arange(T, dtype=np.float32)
    freqs = pos[:, None] * inv_freq[None, :]          # [T, HALF]
    cos_t = np.cos(freqs).T.astype(np.float32)      # [HALF, T]
    sin_t = np.sin(freqs).T.astype(np.float32)
    ccat = np.ascontiguousarray(np.concatenate([cos_t, cos_t], axis=0))
    scat = np.ascontiguousarray(np.concatenate([-sin_t, sin_t], axis=0))

    import ml_dtypes
    tri = np.triu(np.ones((128, 128), dtype=np.float32))  # [k,q]: 1 if k<=q
    tri = tri.astype(ml_dtypes.bfloat16)

    in_maps = []
    for core in range(8):
        b, g = divmod(core, 2)
        xT = np.ascontiguousarray(x[b].T).reshape(NCTILE, 128, T)
        cols = []
        for part in range(3):                          # q, k, v column groups
            base = part * C + g * HPG * D
            cols.append(w_qkv[:, base:base + HPG * D])
        wq = np.ascontiguousarray(np.concatenate(cols, axis=1))
        wpj = np.ascontiguousarray(w_proj[g * HPG * D:(g + 1) * HPG * D, :])
        in_maps.append({
            "xT": xT, "wqkv": wq, "wproj": wpj,
            "ccat": ccat, "scat": scat, "trimask": tri,
        })
    return in_maps


def kernel(x, w_qkv, w_proj):
    nc = _get_program()
    in_maps = _host_prep(x, w_qkv, w_proj)
    res = run_bass_kernel_spmd(nc, in_maps, list(range(8)))

    out = np.empty((B, T, C), dtype=np.float32)
    k = np.empty((B, N_HEAD, T, D), dtype=np.float32)
    v = np.empty((B, N_HEAD, T, D), dtype=np.float32)
    for b in range(B):
        r0 = res.results[2 * b]
        r1 = res.results[2 * b + 1]
        out[b] = r0["out_part"] + r1["out_part"]
        for g, r in ((0, r0), (1, r1)):
            hs = slice(g * HPG, (g + 1) * HPG)
            k[b, hs] = np.transpose(r["k_out"], (0, 2, 1))
            v[b, hs] = np.transpose(r["v_out"], (0, 2, 1))
    return out, k, v
